# revision 45
# baseline (speedup 1.0000x reference)
"""Trainium2 Bass kernel for nn_CGAMotorModel.

Reference computes, for B=512, H=1024, D=5 multivector channels of Cl(4,1):
    W_x[b,h]  = sum_d x[b,d] o W_in[h,d]          (o = geometric product)
    h_free    = (1 - (1-dt)^n) * W_x              (closed form of the scan)
    out[b]    = sum_h h_free[b,h] o W_out[h]

By associativity/bilinearity of the geometric product this collapses to
    out[b] = c * sum_d x[b,d] o K_d,   K_d = sum_h W_in[h,d] o W_out[h]
with c = 1 - 0.9^10 (folded into x on the host, fp32-exact).

H-tensor-parallel over 8 cores (128 H-rows each); host sums the partial
outputs. Per core (all matmul inputs fp16, fp32 PSUM accumulate):
    S[r,(d,q)] = sum_h W_out[h,r] * W_in[h,(d,q)]         (1 matmul, K=128)
    K[g,d]     = sum_q C[q,:,:]^T @ S_q                   (32 matmuls, K=32)
    MT[m,5p+d] = sum_g C[p,g,m] * K[g,d]  per p           (32 matmuls, free=5,
                 psum free-dim offsets; partition base stays 0)
    M[(p,d),m] = PE-transpose of MT (2 transposes)
    out[b,m]   = xt[(p,d),b]^T @ M                        (8 matmuls)
where xt = X^T arrives via two DRAM->SBUF DmaTranspose ops (x host-permuted
to (p,d) column order and zero-padded to 192 cols so the tail window's rows
land at lhsT base partition 64 — matmul requires base 0/32/64), entirely off
the PE critical path. One fp16 Cayley table TBL[r, q*32+g] = C[q,r,g]
(+-1, exact in fp16) serves the K-step (lhsT slabs) and the MT-step
(lhsT slabs, read as TBL[g, 32p+m] = C[p,g,m]).

All matmul inputs are fp16 (1 PE cycle/row vs 4 for fp32); accumulation is
fp32 in PSUM, the free-phase constant c is folded into x on the host in
fp32, and the Cayley tables are exact in fp16, so the only error source is
fp16 rounding of x/W (measured rel err ~4e-4 vs the 2e-2 gate).

All DRAM I/O goes through the SWDGE custom-DMA path instead of dma_start:
inputs via gpsimd.dma_gather (w, tbl as row gathers; X^T via the
transpose-mode gather) and the output via gpsimd.kv_writeback — these skip
the HWDGE descriptor/semaphore latency entirely, so the first matmul starts
~0.6us after kernel start instead of ~2.4us and the store completes in
~0.2us instead of ~2.2us. The gather index ucode reads its 16 channel rows
from different partition groups in CoreSim (0:16) vs walrus BIRSIM (16:32),
so the idx tile repeats (p & 15) + 16s across every 16-partition group
(built with Pool iotas + DVE int32 ALU; integer ALU exists only on DVE).
CoreSim cost-model time: 4461 ns (vs 7527 ns with plain DMAs and 13547 ns
for the DRAM-bounce fp32 baseline).
"""

import numpy as np

import concourse.bass as bass
import concourse.mybir as mybir
import concourse.tile as tile
from concourse import bacc
from concourse.bass_utils import run_bass_kernel_spmd
from concourse.masks import make_identity

B, H, D, MV = 512, 1024, 5, 32
N_CORES = 8
DT, N_FREE = 0.1, 10
C_SCALE = 1.0 - (1.0 - DT) ** N_FREE
F32 = mybir.dt.float32
F16 = mybir.dt.float16
NF16 = np.float16


def _cayley_np() -> np.ndarray:
    """Cayley table for Cl(4,1), metric diag(1,1,1,1,-1). C[a,b,a^b] = sign."""
    metric = np.array([1.0, 1.0, 1.0, 1.0, -1.0], dtype=np.float32)
    C = np.zeros((32, 32, 32), dtype=np.float32)
    for a in range(32):
        for b in range(32):
            cnt = 0
            aa = a >> 1
            while aa:
                cnt += bin(aa & b).count("1")
                aa >>= 1
            s = -1.0 if (cnt & 1) else 1.0
            common = a & b
            for i in range(5):
                if (common >> i) & 1:
                    s *= metric[i]
            C[a, b, a ^ b] = s
    return C


# TBL[r, q*32+g] = C[q,r,g]; equally TBL[g, p*32+m] = C[p,g,m]. Entries are
# in {-1,0,1}: exact in fp16.
TBL = (
    np.ascontiguousarray(_cayley_np().transpose(1, 0, 2)).reshape(32, 1024).astype(NF16)
)

# x column permutation (d,p) -> (p,d): x2[:, 5p+d] = x[:, 32d+p]
PERM = np.array([32 * d + p for p in range(32) for d in range(5)], dtype=np.int64)




def build_program() -> bass.Bass:
    # Bacc (not plain Bass): its compile pass moves multi-sem matmul waits
    # onto LdWeights — walrus rejects Matmult with >1 sync wait otherwise.
    nc = bacc.Bacc()
    x2 = nc.dram_tensor("x2", [512, 256], F16, kind="ExternalInput")
    # wcat = [W_in.reshape(128,160) | W_out.reshape(128,32) | zero pad] for
    # this core's 128-row H-chunk; 256-col rows keep the DMA descriptor
    # elements at 512B, dodging the sub-512B 2x DMA latency penalty
    wcat = nc.dram_tensor("wcat", [128, 256], F16, kind="ExternalInput")
    tbl = nc.dram_tensor("tbl", [32, 1024], F16, kind="ExternalInput")
    # native layout [p, (t m)], b = 128t + p — host de-interleaves
    out = nc.dram_tensor("out", [128, 4 * MV], F32, kind="ExternalOutput")

    with tile.TileContext(nc) as tc:
        with (
            tc.tile_pool(name="sb", bufs=1) as sb,
            tc.tile_pool(name="ps", bufs=1, space="PSUM") as ps,
        ):
            # --- All standard-library gpsimd ops first (iota/memset/ident),
            # then the custom-library SWDGE ops (gathers, writebacks): the
            # framework wraps each library switch in an all-engine barrier
            # that drains in-flight DMAs, so the switch must happen once,
            # early, with nothing in flight.
            # one idx tile serves all three gathers: value = (p & 15) + 16s,
            # identical across every 16-partition group — the ucode reads its
            # channel rows from different partition groups in CoreSim vs
            # BIRSIM, so the content must repeat. The w/tbl index sets are
            # column-prefixes of the x index set.
            pidx = sb.tile([128, 32], mybir.dt.int32, tag="pidx")
            sidx = sb.tile([128, 32], mybir.dt.int32, tag="sidx")
            nc.gpsimd.iota(pidx[:], [[0, 32]], base=0, channel_multiplier=1)
            nc.gpsimd.iota(sidx[:], [[16, 32]], base=0, channel_multiplier=0)
            # integer ALU only exists on DVE (32-bit); narrow to i16 on write
            nc.vector.tensor_scalar(
                pidx[:], pidx[:], 15, None, mybir.AluOpType.bitwise_and
            )
            nc.vector.tensor_add(pidx[:], pidx[:], sidx[:])
            xidx = sb.tile([128, 32], mybir.dt.int16, tag="xidx")
            nc.vector.tensor_copy(xidx[:], pidx[:])
            # --- SWDGE gathers: w (gates S), tbl (gates K), x^T (gates the
            # final). A row-gather costs ~free_size cycles on Pool with no
            # HWDGE descriptor/semaphore latency.
            w_sb = sb.tile([128, 256], F16, tag="w_sb")
            nc.gpsimd.dma_gather(
                w_sb[:].rearrange("p (u f) -> p u f", u=1),
                wcat[:],
                xidx[:, 0:8],
                num_idxs=128,
                num_idxs_reg=128,
                elem_size=256,
            )
            # gather dst must be 128-partition; rows 32:128 stay unused
            tblg = sb.tile([128, 1024], F16, tag="tblg")
            nc.gpsimd.dma_gather(
                tblg[:].rearrange("p (u f) -> p u f", u=1),
                tbl[:],
                xidx[:, 0:2],
                num_idxs=32,
                num_idxs_reg=32,
                elem_size=1024,
            )
            # deferred std-lib Pool ops: nothing below gates the S/K chain
            # identity for the PE transposes of MT (generated on Pool, no DMA)
            ident_sb = sb.tile([32, 32], F16, tag="ident_sb")
            make_identity(nc, ident_sb[:])
            # zero ctx indices for the kv_writeback output store
            ctxz = sb.tile([128, 128], mybir.dt.int32, tag="ctxz")
            nc.gpsimd.memset(ctxz[:], 0)
            # mts zero tail (see MT-step below)
            mts = sb.tile([32, 192], F16, tag="mts")
            nc.gpsimd.memset(mts[:, 160:192], 0.0)
            # transpose-mode gather: xtv[j, c, b] = x2[b, 128c+j] — X^T
            # straight from DRAM. Host placed pd cols 0:128 at x2 cols 0:128
            # (-> c=0) and the pd tail 128:160 at cols 192:224 (-> c=1 rows
            # 64:96, matching the matmul base-64 alignment of m2).
            xtv = sb.tile([128, 2, B], F16, tag="xtv")
            nc.gpsimd.dma_gather(
                xtv[:],
                x2[:],
                xidx[:],
                num_idxs=B,
                num_idxs_reg=B,
                elem_size=256,
                transpose=True,
            )

            # --- S-step: one matmul, K=128 H-rows ---
            spsum = ps.tile([32, 160], F32, tag="spsum")
            nc.tensor.matmul(
                spsum[:], w_sb[:, 160:192], w_sb[:, 0:160], start=True, stop=True
            )
            ssb = sb.tile([32, 160], F16, tag="ssb")
            nc.vector.tensor_copy(ssb[:], spsum[:])

            # --- K-step: K[g,d] = sum_q C[q]^T @ S_q ---
            kpsum = ps.tile([32, D], F32, tag="kpsum")
            for q in range(32):
                nc.tensor.matmul(
                    kpsum[:],
                    tblg[0:32, 32 * q : 32 * (q + 1)],
                    ssb[:, q : 160 : 32],
                    start=(q == 0),
                    stop=(q == 31),
                )
            ksb = sb.tile([32, D], F16, tag="ksb")
            nc.vector.tensor_copy(ksb[:], kpsum[:])

            # --- MT-step: MT[m, 5p+d] = sum_g C[p,g,m] * K[g,d] per p, at psum
            # FREE offset 5p (matmul psum partition base must be 0/32/64, so
            # the transposed layout with per-p free offsets is the legal one).
            mtp = ps.tile([32, 160], F32, tag="mtp")
            for p in range(32):
                nc.tensor.matmul(
                    mtp[:, 5 * p : 5 * p + 5],
                    tblg[0:32, 32 * p : 32 * (p + 1)],
                    ksb[:],
                    start=True,
                    stop=True,
                )
            # mts padded to 192 cols (Pool zeroes the tail at t~0) so the
            # second transpose below can read a full 128-wide window ending
            # at col 192 — its output then covers every m12p row, and the
            # single m12 copy never reads uninitialized PSUM.
            nc.vector.tensor_copy(mts[:, 0:160], mtp[:])

            # --- M = MT^T via two PE transposes ---
            # Both transposes land in ONE fp16 psum tile (the tail at
            # partition base 64 to match xtb's live rows — matmul requires
            # lhsT and rhs to share their base partition), so a single DVE
            # copy (2-byte 2x mode) moves M to SBUF.
            m12p = ps.tile([128, 2 * MV], F16, tag="m12p")
            nc.tensor.transpose(m12p[:, 0:MV], mts[:, 0:128], ident_sb[:])
            # full-height window: rows 64:96 of this output are mts cols
            # 128:160, i.e. M rows 128:160; other rows are live-but-unused
            nc.tensor.transpose(m12p[:, MV : 2 * MV], mts[:, 64:192], ident_sb[:])
            m12 = sb.tile([128, 2 * MV], F16, tag="m12")
            nc.vector.tensor_copy(m12[:], m12p[:])

            # --- final: out[b,m], 4 row-blocks of 128, all into ONE psum
            # bank (free-offset accumulation groups) -> ONE osb copy ---
            op = ps.tile([128, 4 * MV], F32, tag="opsum")
            for t in range(4):
                nc.tensor.matmul(
                    op[:, MV * t : MV * (t + 1)],
                    xtv[:, 0, 128 * t : 128 * (t + 1)],
                    m12[:, 0:MV],
                    start=True,
                    stop=False,
                )
                nc.tensor.matmul(
                    op[:, MV * t : MV * (t + 1)],
                    xtv[64:96, 1, 128 * t : 128 * (t + 1)],
                    m12[64:96, MV : 2 * MV],
                    start=False,
                    stop=True,
                )
            osb = sb.tile([128, 4 * MV], F32, tag="osb")
            nc.vector.tensor_copy(osb[:], op[:])
            # output via kv_writeback (SWDGE token store): out[f, p] =
            # osb[p, f] — the transposed layout is undone on the host. Two
            # batch-halves keep each call under the descriptor carveout.
            for h in range(2):
                nc.gpsimd.kv_writeback(
                    out[64 * h : 64 * (h + 1), :].rearrange(
                        "b (i o n) -> b i o n", o=1, n=1
                    ),
                    osb[:, 64 * h : 64 * (h + 1)].rearrange(
                        "p (o b n) -> p o b n", o=1, n=1
                    ),
                    ctxz[:, 0:64],
                )

    nc.finalize()
    return nc


_NC_CACHE: list = []


def make_inputs(x_mv: np.ndarray, W_in: np.ndarray, W_out: np.ndarray):
    """Host-side marshaling: fold c into x, permute columns to (p,d) order,
    cast matmul inputs to fp16, slice per-core H-chunks."""
    x = np.asarray(x_mv, dtype=np.float32)
    Wi = np.asarray(W_in, dtype=np.float32)
    Wo = np.asarray(W_out, dtype=np.float32)
    x2 = np.zeros((B, 256), dtype=NF16)
    xp = (C_SCALE * x.reshape(B, D * MV))[:, PERM].astype(NF16)
    x2[:, 0:128] = xp[:, 0:128]
    x2[:, 192:224] = xp[:, 128:160]
    wcat = np.zeros((H, 256), dtype=NF16)
    wcat[:, 0 : D * MV] = Wi.reshape(H, D * MV).astype(NF16)
    wcat[:, D * MV : D * MV + MV] = Wo.reshape(H, MV).astype(NF16)
    return [
        {"x2": x2, "wcat": wcat.reshape(N_CORES, 128, 256)[c], "tbl": TBL}
        for c in range(N_CORES)
    ]


def kernel(x_mv: np.ndarray, W_in: np.ndarray, W_out: np.ndarray) -> np.ndarray:
    if not _NC_CACHE:
        _NC_CACHE.append(build_program())
    nc = _NC_CACHE[0]

    in_maps = make_inputs(x_mv, W_in, W_out)
    try:
        res = run_bass_kernel_spmd(nc, in_maps, core_ids=list(range(N_CORES)))
    except Exception:
        # transient NRT/device hiccups have been observed; one retry
        res = run_bass_kernel_spmd(nc, in_maps, core_ids=list(range(N_CORES)))
    parts = [res.results[c]["out"] for c in range(N_CORES)]
    # kv_writeback stores transposed: out[(t m), p]; de-interleave to
    # b = 128t + p
    out = np.sum(parts, axis=0, dtype=np.float32).reshape(4, MV, 128)
    out = out.transpose(0, 2, 1).reshape(B, MV)
    return np.ascontiguousarray(out, dtype=np.float32).reshape(B, 1, MV)
